# revision 23
# baseline (speedup 1.0000x reference)
"""GNN message-passing kernel for trn2: preprocessing + bass/tile builder.

v2: quad-aligned node numbering so the inter-layer AllGather can be fired in
4 window-chunks and overlap layer-2 gathers; quad-outer aggregation with an
SBUF f32 accumulator; per-call msg tiles to decouple descriptor generation
from DMA drains.
"""
import numpy as np
import ml_dtypes
import concourse.bass as bass
import concourse.tile as tile
from concourse import bacc, mybir
from concourse.bass_utils import run_bass_kernel_spmd

F32 = mybir.dt.float32
BF16 = mybir.dt.bfloat16
I16 = mybir.dt.int16
P = 128
NCHUNK = 4  # AllGather chunks == gather quads


def preprocess(x, edge_index, batch, NC=8, table_np=ml_dtypes.bfloat16):
    """Host-side graph preprocessing. Returns (struct, per_core_common, meta)."""
    x = np.asarray(x, np.float32)
    ei = np.asarray(edge_index, np.int64)
    b = np.asarray(batch, np.int64)
    N = x.shape[0]
    G = int(b.max()) + 1
    assert G % NC == 0, (G, NC)
    GPC = G // NC
    counts = np.bincount(b, minlength=G)
    assert counts.min() > 0
    WPG = int(np.ceil(counts.max() / P))  # windows per graph
    NPG = WPG * P
    WIN = GPC * WPG                       # windows per core
    assert WIN % NCHUNK == 0
    CW = WIN // NCHUNK                    # windows per chunk
    NPC = WIN * P                         # padded nodes per core
    NTOT = NC * NPC
    QR = NC * CW * P                      # rows per quad/chunk
    assert QR <= 32767, QR                # int16 gather index range
    NQ = NCHUNK

    # node numbering: graph g -> core g//GPC, window w (core-local), slot d.
    # global row id = (w//CW)*QR + c*(CW*P) + (w%CW)*P + d  (quad-major) so an
    # AllGather of all cores' windows [q*CW,(q+1)*CW) lands as quad q.
    cum = np.concatenate([[0], np.cumsum(counts)])
    gidx = np.arange(G)
    core_of_g = gidx // GPC
    w0_of_g = (gidx % GPC) * WPG          # first core-local window of graph

    # per-node: core, local window, slot
    node_rank = np.arange(N) - cum[b]     # position within graph
    w_loc = w0_of_g[b] + node_rank // P   # core-local window 0..WIN-1
    d_loc = node_rank % P
    c_of = core_of_g[b]
    perm = (w_loc // CW) * QR + c_of * (CW * P) + (w_loc % CW) * P + d_loc

    xt = np.zeros((NTOT, x.shape[1]), table_np)
    xt[perm] = x.astype(table_np)

    src = perm[ei[0]]
    dst = perm[ei[1]]
    deg = np.bincount(dst, minlength=NTOT)

    # decompose dst global id -> (core, window, lane)
    def decomp(g):
        q = g // QR
        r = g % QR
        c = r // (CW * P)
        w = q * CW + (r % (CW * P)) // P
        d = r % P
        return c, w, d

    c_d, w_d, d_l = decomp(dst)
    q_s = src // QR
    idx_loc = src % QR

    # recip / mask in (core, lane, window) layout
    recip_pc = np.zeros((NC, P, WIN), np.float32)
    mask_pc = np.zeros((NC, 1, NPC), table_np)
    gl = (np.arange(NTOT) // QR) * 0  # placeholder
    # build from deg via decomposition of all rows
    allg = np.arange(NTOT)
    ac, aw, ad = decomp(allg)
    recip_full = (1.0 / np.maximum(deg, 1)).astype(np.float32)
    mask_full = (deg > 0).astype(np.float32)
    recip_pc[ac, ad, aw] = recip_full
    mask_pc[ac, 0, aw * P + ad] = mask_full.astype(table_np)

    # group-pooled slot layout: per (grp of GRP windows, q) the windows' exact
    # max-over-core counts are packed back to back and only the group total is
    # rounded up to 128. Subtiles may straddle window boundaries; each
    # (subtile, window) segment gets its own dst column / matmul.
    cnt = np.zeros((NC, WIN, NQ), np.int64)
    np.add.at(cnt, (c_d, w_d, q_s), 1)
    M = cnt.max(axis=0)                                  # [WIN, NQ] slots/window
    GRP = 8
    NG = WIN // GRP
    woff = np.zeros((WIN, NQ), np.int64)   # slot offset of w within its (g,q)
    NSUB = np.zeros((NG, NQ), np.int64)    # subtiles per (g,q)
    base = np.zeros((NG, NQ), np.int64)    # first (global) subtile of (g,q)
    segs = {}                              # (g,q) -> [(s_loc, w, col, a, b)]
    run = 0
    nseg = 0
    for q in range(NQ):
        for g in range(NG):
            off = 0
            for w in range(g * GRP, (g + 1) * GRP):
                woff[w, q] = off
                off += int(M[w, q])
            ns = (off + P - 1) // P
            NSUB[g, q] = ns
            base[g, q] = run
            run += ns
            lst = []
            for w in range(g * GRP, (g + 1) * GRP):
                lo = int(woff[w, q])
                hi = lo + int(M[w, q])
                if hi == lo:
                    continue
                for s in range(lo // P, (hi - 1) // P + 1):
                    a = max(lo, s * P)
                    b = min(hi, (s + 1) * P)
                    lst.append((s, w, nseg, a, b))
                    nseg += 1
            segs[(g, q)] = lst
    S_tot = run
    NSEG = nseg

    # sort edges by (core, q_src, window) then assign slots
    key = ((c_d * NQ + q_s) * WIN + w_d)
    order = np.argsort(key, kind="stable")
    s_idx = idx_loc[order]
    s_dl = d_l[order]
    s_key = key[order]
    L = np.bincount(s_key, minlength=NC * NQ * WIN).reshape(NC, NQ, WIN)

    idx_flat = np.zeros((NC, S_tot * P), np.int64)       # pad -> row 0
    dl_flat = np.full((NC, S_tot * P), 255, np.int64)    # pad -> dead dst
    grp_start = np.concatenate([[0], np.cumsum(L.reshape(-1))])
    for c in range(NC):
        for q in range(NQ):
            for w in range(WIN):
                gi = (c * NQ + q) * WIN + w
                n = L[c, q, w]
                if n == 0:
                    continue
                a = grp_start[gi]
                sl0 = int(base[w // GRP, q]) * P + int(woff[w, q])
                idx_flat[c, sl0:sl0 + n] = s_idx[a:a + n]
                dl_flat[c, sl0:sl0 + n] = s_dl[a:a + n]
    assert idx_flat.max() < QR and idx_flat.min() >= 0

    idx16 = np.ascontiguousarray(
        idx_flat.reshape(NC, S_tot * 8, 16).transpose(0, 2, 1)).astype(np.int16)
    idx16 = np.tile(idx16, (1, 8, 1))                    # [NC, 128, S_tot*8]

    # per-segment dst columns [NC, NSEG, P], 255 outside the segment range
    dst_cols = np.full((NC, NSEG, P), 255, np.int64)
    for (g, q), lst in segs.items():
        b0 = int(base[g, q]) * P
        for (s, w, col, a, b) in lst:
            dst_cols[:, col, a - s * P:b - s * P] = dl_flat[:, b0 + a:b0 + b]
    dst_arr = np.ascontiguousarray(
        dst_cols.transpose(0, 2, 1)).astype(ml_dtypes.bfloat16)

    struct = dict(NC=NC, G=G, GPC=GPC, WPG=WPG, WIN=WIN, NPC=NPC, NTOT=NTOT,
                  NQ=NQ, QR=QR, CW=CW, S_tot=S_tot, NSEG=NSEG,
                  NSUB=NSUB, base=base, segs=segs, woff=woff, M=M,
                  GRP=GRP, NG=NG)
    percore = dict(idx16=idx16, dst=dst_arr, recip=recip_pc, mask=mask_pc, xt=xt)
    meta = dict(S_tot=S_tot, WPG=WPG)
    return struct, percore, meta


def build_nc(st, D=128, OUT=2, table_dt=BF16):
    NC, WIN, NPC, NTOT = st["NC"], st["WIN"], st["NPC"], st["NTOT"]
    NQ, QR, CW, GPC, WPG = st["NQ"], st["QR"], st["CW"], st["GPC"], st["WPG"]
    S_tot, NSEG, GRP, NG = st["S_tot"], st["NSEG"], st["GRP"], st["NG"]
    NSUB, base, segs = st["NSUB"], st["base"], st["segs"]
    DT = BF16

    nc = bacc.Bacc("TRN2", target_bir_lowering=False, debug=False,
                   num_devices=NC, num_swdge_queues=4,
                   dynamic_dma_scratch_size=16384)
    xt = nc.dram_tensor("xt", [NTOT, D], table_dt, kind="ExternalInput")
    idx_in = nc.dram_tensor("idx16", [P, S_tot * 8], I16, kind="ExternalInput")
    dst_in = nc.dram_tensor("dstl", [P, NSEG], BF16, kind="ExternalInput")
    recip_in = nc.dram_tensor("recip", [P, WIN], F32, kind="ExternalInput")
    mask_in = nc.dram_tensor("mask", [1, NPC], BF16, kind="ExternalInput")
    w1t_in = nc.dram_tensor("w1t", [D, D], DT, kind="ExternalInput")
    w2t_in = nc.dram_tensor("w2t", [D, D], DT, kind="ExternalInput")
    b1r_in = nc.dram_tensor("b1r", [1, D], DT, kind="ExternalInput")
    b2r_in = nc.dram_tensor("b2r", [1, D], DT, kind="ExternalInput")
    wf1t_in = nc.dram_tensor("wf1t", [D, D], F32, kind="ExternalInput")
    bf1r_in = nc.dram_tensor("bf1r", [1, D], F32, kind="ExternalInput")
    wf2t_in = nc.dram_tensor("wf2t", [D, OUT], F32, kind="ExternalInput")
    bf2r_in = nc.dram_tensor("bf2r", [1, OUT], F32, kind="ExternalInput")
    iota_in = nc.dram_tensor("iota", [P, P], BF16, kind="ExternalInput")
    identb_in = nc.dram_tensor("identb", [P, P], BF16, kind="ExternalInput")
    identf_in = nc.dram_tensor("identf", [P, P], F32, kind="ExternalInput")
    onesg_in = nc.dram_tensor("onesg", [1, NC * GPC], F32, kind="ExternalInput")
    out = nc.dram_tensor("out", [NC * GPC, OUT], F32, kind="ExternalOutput")
    h_loc = nc.dram_tensor("h_loc", [NPC, D], table_dt, kind="Internal")
    htq = [nc.dram_tensor(f"htq{q}", [QR, D], table_dt, kind="Internal",
                          addr_space="Shared") for q in range(NQ)]

    with tile.TileContext(nc) as tc:
        cp = tc.alloc_tile_pool(name="const", bufs=1)
        wp = tc.alloc_tile_pool(name="work", bufs=3)
        mp = tc.alloc_tile_pool(name="msgs", bufs=2)
        ohp = tc.alloc_tile_pool(name="ohp", bufs=4)
        pp_agg = tc.alloc_tile_pool(name="ps_agg", bufs=2, space="PSUM")
        pp_t = tc.alloc_tile_pool(name="ps_t", bufs=2, space="PSUM")
        pp_h = tc.alloc_tile_pool(name="ps_h", bufs=2, space="PSUM")
        pp_p = tc.alloc_tile_pool(name="ps_p", bufs=2, space="PSUM")

        idx_t = cp.tile([P, S_tot * 8], I16)
        nc.sync.dma_start(idx_t[:], idx_in[:])
        dst_t = cp.tile([P, NSEG], BF16)
        nc.sync.dma_start(dst_t[:], dst_in[:])
        recip_t = cp.tile([P, WIN], F32)
        nc.sync.dma_start(recip_t[:], recip_in[:])
        mask_t = cp.tile([1, NPC], BF16)
        nc.sync.dma_start(mask_t[:], mask_in[:])
        w1t_t = cp.tile([D, D], DT)
        nc.sync.dma_start(w1t_t[:], w1t_in[:])
        w2t_t = cp.tile([D, D], DT)
        nc.sync.dma_start(w2t_t[:], w2t_in[:])
        b1r_t = cp.tile([1, D], DT)
        nc.sync.dma_start(b1r_t[:], b1r_in[:])
        b2r_t = cp.tile([1, D], DT)
        nc.sync.dma_start(b2r_t[:], b2r_in[:])
        wf1t_t = cp.tile([D, D], F32)
        nc.sync.dma_start(wf1t_t[:], wf1t_in[:])
        bf1r_t = cp.tile([1, D], F32)
        nc.sync.dma_start(bf1r_t[:], bf1r_in[:])
        wf2t_t = cp.tile([D, OUT], F32)
        nc.sync.dma_start(wf2t_t[:], wf2t_in[:])
        bf2r_t = cp.tile([1, OUT], F32)
        nc.sync.dma_start(bf2r_t[:], bf2r_in[:])
        iota_t = cp.tile([P, P], BF16)
        nc.sync.dma_start(iota_t[:], iota_in[:])
        identb_t = cp.tile([P, P], BF16)
        nc.sync.dma_start(identb_t[:], identb_in[:])
        identf_t = cp.tile([P, P], F32)
        nc.sync.dma_start(identf_t[:], identf_in[:])
        onesg_t = cp.tile([1, NC * GPC], F32)
        nc.sync.dma_start(onesg_t[:], onesg_in[:])
        poolT = cp.tile([P, GPC], F32)
        nc.vector.memset(poolT[:], 0)
        agg_sb = cp.tile([P, WIN * D], F32)  # f32 accumulator, all windows

        Sg_max = int(NSUB.max())
        MAXSUB = 8  # <=1024 idx per call (swdge scratch limit)

        callno = 0
        for layer in range(2):
            wt = w1t_t if layer == 0 else w2t_t
            br = b1r_t if layer == 0 else b2r_t
            nc.vector.memset(agg_sb[:], 0)
            next_ag = 0
            for g in range(NG):
                ws = list(range(g * GRP, (g + 1) * GRP))
                for q in range(NQ):
                    table = xt[q * QR:(q + 1) * QR, :] if layer == 0 else htq[q][:, :]
                    Sg = int(NSUB[g, q])
                    if Sg == 0:
                        continue
                    base_sub = int(base[g, q])
                    ncall = (Sg + MAXSUB - 1) // MAXSUB
                    lo_n = Sg // ncall
                    sizes = [lo_n + (1 if j < Sg % ncall else 0)
                             for j in range(ncall)]
                    tiles = []
                    off = 0
                    for j, n in enumerate(sizes):
                        mt = mp.tile([P, n * D], table_dt,
                                     tag=f"msgs{callno % 8}",
                                     name=f"m{layer}_{q}_{g}_{j}",
                                     padded_shape=[P, MAXSUB * D])
                        tiles.append((mt, off, n))
                        c0 = (base_sub + off) * 8
                        nc.gpsimd.dma_gather(
                            out_ap=mt[:].rearrange("p (s d) -> p s d", d=D),
                            in_ap=table,
                            idxs_ap=idx_t[:, c0: c0 + n * 8],
                            num_idxs=n * P,
                            num_idxs_reg=n * P,
                            elem_size=D,
                            queue_num=callno % 4,
                        )
                        callno += 1
                        off += n

                    def m3_of(s):
                        for mt, o, n in tiles:
                            if o <= s < o + n:
                                return mt[:].rearrange(
                                    "p (s d) -> p s d", d=D)[:, s - o, :]
                        raise AssertionError(s)

                    for w in ws:
                        segl = [t for t in segs[(g, q)] if t[1] == w]
                        if not segl:
                            continue
                        ps = pp_agg.tile([P, D], F32, tag="agg")
                        for k, (s, _w, col, a, b) in enumerate(segl):
                            oh = ohp.tile([P, P], BF16, tag="oh")
                            nc.vector.tensor_tensor(
                                out=oh[:],
                                in0=dst_t[:, col:col + 1].to_broadcast([P, P]),
                                in1=iota_t[:],
                                op=mybir.AluOpType.is_equal,
                            )
                            nc.tensor.matmul(ps[:], lhsT=oh[:], rhs=m3_of(s),
                                             start=(k == 0),
                                             stop=(k == len(segl) - 1))
                        aslice = agg_sb[:, w * D:(w + 1) * D]
                        nc.vector.tensor_tensor(out=aslice, in0=ps[:], in1=aslice,
                                                op=mybir.AluOpType.add)

                # transforms for this group's windows (all quads aggregated);
                # layer0 additionally fires AllGather chunks as soon as all
                # windows of a chunk have been written to h_loc.
                for w in ws:
                    agg_s = wp.tile([P, D], BF16, tag="aggs")
                    nc.vector.tensor_scalar(out=agg_s[:],
                                            in0=agg_sb[:, w * D:(w + 1) * D],
                                            scalar1=recip_t[:, w:w + 1],
                                            scalar2=None,
                                            op0=mybir.AluOpType.mult)
                    aggT_p = pp_t.tile([P, D], BF16, tag="aggT")
                    nc.tensor.transpose(aggT_p[:], agg_s[:], identb_t[:])
                    aggT_s = wp.tile([P, D], BF16, tag="aggTs")
                    nc.scalar.activation(aggT_s[:], aggT_p[:],
                                         mybir.ActivationFunctionType.Copy)
                    h_p = pp_h.tile([P, D], F32, tag="h")
                    nc.tensor.matmul(h_p[:], lhsT=aggT_s[:], rhs=wt[:],
                                     start=True, stop=False)
                    nc.tensor.matmul(h_p[:], lhsT=mask_t[:1, w * P:(w + 1) * P],
                                     rhs=br[:], start=False, stop=True)
                    h_s = wp.tile([P, D], table_dt, tag="hs")
                    nc.scalar.activation(h_s[:], h_p[:],
                                         mybir.ActivationFunctionType.Relu)
                    if layer == 0:
                        nc.sync.dma_start(h_loc[w * P:(w + 1) * P, :], h_s[:])
                    else:
                        lg = w // WPG
                        hT_p = pp_p.tile([P, P], BF16, tag="hT")
                        nc.tensor.transpose(hT_p[:], h_s[:], identb_t[:])
                        wmax = wp.tile([P, 1], F32, tag="wmax")
                        nc.vector.reduce_max(wmax[:], hT_p[:],
                                             axis=mybir.AxisListType.X)
                        nc.vector.tensor_tensor(out=poolT[:, lg:lg + 1],
                                                in0=wmax[:],
                                                in1=poolT[:, lg:lg + 1],
                                                op=mybir.AluOpType.max)
                if layer == 0:
                    while (next_ag < NCHUNK
                           and ws[-1] >= (next_ag + 1) * CW - 1):
                        nc.gpsimd.collective_compute(
                            "AllGather", mybir.AluOpType.bypass,
                            replica_groups=[list(range(NC))],
                            ins=[h_loc[next_ag * CW * P:(next_ag + 1) * CW * P, :]],
                            outs=[htq[next_ag][:]],
                        )
                        next_ag += 1

        # ---- head ----
        NGr = NC * GPC
        dp = tc.alloc_tile_pool(name="dram", bufs=1, space="DRAM")
        pag_in = dp.tile([P, GPC], F32)
        pag_out = dp.tile([NC * P, GPC], F32, addr_space="Shared")
        nc.sync.dma_start(pag_in[:], poolT[:])
        nc.gpsimd.collective_compute(
            "AllGather", mybir.AluOpType.bypass,
            replica_groups=[list(range(NC))],
            ins=[pag_in[:]], outs=[pag_out[:]],
        )
        pall = cp.tile([P, NGr], F32)
        pr = pag_out[:].rearrange("(c p) g -> p c g", c=NC)
        for c in range(NC):
            nc.sync.dma_start(pall[:, c * GPC:(c + 1) * GPC], pr[:, c, :])
        z_p = pp_agg.tile([P, NGr], F32, tag="agg")
        nc.tensor.matmul(z_p[:], lhsT=wf1t_t[:], rhs=pall[:], start=True, stop=False)
        nc.tensor.matmul(z_p[:], lhsT=bf1r_t[:1, :], rhs=onesg_t[:1, :],
                         start=False, stop=True)
        zr = wp.tile([P, NGr], F32, tag="zr")
        nc.scalar.activation(zr[:], z_p[:], mybir.ActivationFunctionType.Relu)
        z2_p = pp_h.tile([OUT, NGr], F32, tag="h")
        nc.tensor.matmul(z2_p[:], lhsT=wf2t_t[:], rhs=zr[:], start=True, stop=False)
        nc.tensor.matmul(z2_p[:], lhsT=bf2r_t[:1, :], rhs=onesg_t[:1, :],
                         start=False, stop=True)
        z2 = wp.tile([OUT, NGr], F32, tag="z2")
        nc.vector.tensor_copy(z2[:], z2_p[:])
        zt_p = pp_t.tile([NGr, OUT], F32, tag="aggT")
        nc.tensor.transpose(zt_p[:], z2[:], identf_t[:OUT, :OUT])
        zt = wp.tile([NGr, OUT], F32, tag="zt")
        nc.vector.tensor_copy(zt[:], zt_p[:])
        mx = wp.tile([NGr, 1], F32, tag="mx")
        nc.vector.reduce_max(mx[:], zt[:], axis=mybir.AxisListType.X)
        zs = wp.tile([NGr, OUT], F32, tag="zs")
        nc.vector.tensor_scalar(out=zs[:], in0=zt[:], scalar1=mx[:], scalar2=None,
                                op0=mybir.AluOpType.subtract)
        ex = wp.tile([NGr, OUT], F32, tag="ex")
        nc.scalar.activation(ex[:], zs[:], mybir.ActivationFunctionType.Exp)
        sm = wp.tile([NGr, 1], F32, tag="sm")
        nc.vector.reduce_sum(sm[:], ex[:], axis=mybir.AxisListType.X)
        lg_ = wp.tile([NGr, 1], F32, tag="lg")
        nc.scalar.activation(lg_[:], sm[:], mybir.ActivationFunctionType.Ln)
        logz = wp.tile([NGr, 1], F32, tag="logz")
        nc.vector.tensor_tensor(out=logz[:], in0=mx[:], in1=lg_[:],
                                op=mybir.AluOpType.add)
        res = wp.tile([NGr, OUT], F32, tag="res")
        nc.vector.tensor_scalar(out=res[:], in0=zt[:], scalar1=logz[:], scalar2=None,
                                op0=mybir.AluOpType.subtract)
        nc.sync.dma_start(out[:], res[:])

        for p_ in (dp, pp_p, pp_h, pp_t, pp_agg, ohp, mp, wp, cp):
            p_.release()
    nc.compile()
    return nc


def make_inputs(st, percore, W1, b1, W2, b2, Wf1, bf1, Wf2, bf2):
    NC, GPC = st["NC"], st["GPC"]
    bf = ml_dtypes.bfloat16
    iota = np.broadcast_to(np.arange(P, dtype=np.float32), (P, P)).astype(bf)
    ident = np.eye(P, dtype=np.float32)
    common = dict(
        xt=percore["xt"],
        w1t=np.ascontiguousarray(np.asarray(W1, np.float32).T).astype(bf),
        w2t=np.ascontiguousarray(np.asarray(W2, np.float32).T).astype(bf),
        b1r=np.asarray(b1, np.float32)[None, :].astype(bf),
        b2r=np.asarray(b2, np.float32)[None, :].astype(bf),
        wf1t=np.ascontiguousarray(np.asarray(Wf1, np.float32).T),
        bf1r=np.asarray(bf1, np.float32)[None, :],
        wf2t=np.ascontiguousarray(np.asarray(Wf2, np.float32).T),
        bf2r=np.asarray(bf2, np.float32)[None, :],
        iota=np.ascontiguousarray(iota),
        identb=ident.astype(bf),
        identf=ident,
        onesg=np.ones((1, NC * GPC), np.float32),
    )
    in_maps = []
    for c in range(NC):
        m = dict(common)
        m["idx16"] = np.ascontiguousarray(percore["idx16"][c])
        m["dstl"] = np.ascontiguousarray(percore["dst"][c])
        m["recip"] = np.ascontiguousarray(percore["recip"][c])
        m["mask"] = np.ascontiguousarray(percore["mask"][c])
        in_maps.append(m)
    return in_maps


_CACHE = {}


def kernel(**inputs):
    """Full-input GNN kernel: shards across 8 NeuronCores internally."""
    import os
    x = np.asarray(inputs["x"], np.float32)
    ei = np.asarray(inputs["edge_index"])
    batch = np.asarray(inputs["batch"])
    st, percore, _meta = preprocess(x, ei, batch)
    key = (st["WIN"], st["NPC"], st["S_tot"], st["NQ"])
    if key not in _CACHE:
        _CACHE[key] = build_nc(st)
    nc = _CACHE[key]
    in_maps = make_inputs(st, percore,
                          inputs["W1"], inputs["b1"], inputs["W2"], inputs["b2"],
                          inputs["Wf1"], inputs["bf1"], inputs["Wf2"], inputs["bf2"])
    trace = os.environ.get("GNN_TRACE", "0") == "1"
    res = run_bass_kernel_spmd(nc, in_maps, core_ids=list(range(st["NC"])), trace=trace)
    global LAST_EXEC_NS, LAST_TRACE
    LAST_EXEC_NS = res.exec_time_ns
    LAST_TRACE = res.instructions_and_trace[1] if res.instructions_and_trace else None
    return np.asarray(res.results[0]["out"], np.float32)


LAST_EXEC_NS = None
LAST_TRACE = None


# revision 28
# speedup vs baseline: 1.1254x; 1.1254x over previous
"""GNN message-passing kernel for trn2: preprocessing + bass/tile builder.

v2: quad-aligned node numbering so the inter-layer AllGather can be fired in
4 window-chunks and overlap layer-2 gathers; quad-outer aggregation with an
SBUF f32 accumulator; per-call msg tiles to decouple descriptor generation
from DMA drains.
"""
import numpy as np
import ml_dtypes
import concourse.bass as bass
import concourse.tile as tile
from concourse import bacc, mybir
from concourse.bass_utils import run_bass_kernel_spmd

F32 = mybir.dt.float32
BF16 = mybir.dt.bfloat16
I16 = mybir.dt.int16
P = 128
NCHUNK = 4  # AllGather chunks == gather quads


def preprocess(x, edge_index, batch, NC=8, table_np=ml_dtypes.bfloat16):
    """Host-side graph preprocessing. Returns (struct, per_core_common, meta)."""
    x = np.asarray(x, np.float32)
    ei = np.asarray(edge_index, np.int64)
    b = np.asarray(batch, np.int64)
    N = x.shape[0]
    G = int(b.max()) + 1
    assert G % NC == 0, (G, NC)
    GPC = G // NC
    counts = np.bincount(b, minlength=G)
    assert counts.min() > 0
    WPG = int(np.ceil(counts.max() / P))  # windows per graph
    NPG = WPG * P
    WIN = GPC * WPG                       # windows per core
    assert WIN % NCHUNK == 0
    CW = WIN // NCHUNK                    # windows per chunk
    NPC = WIN * P                         # padded nodes per core
    NTOT = NC * NPC
    QR = NC * CW * P                      # rows per quad/chunk
    assert QR <= 32767, QR                # int16 gather index range
    NQ = NCHUNK

    # node numbering: graph g -> core g//GPC, window w (core-local), slot d.
    # global row id = (w//CW)*QR + c*(CW*P) + (w%CW)*P + d  (quad-major) so an
    # AllGather of all cores' windows [q*CW,(q+1)*CW) lands as quad q.
    cum = np.concatenate([[0], np.cumsum(counts)])
    gidx = np.arange(G)
    core_of_g = gidx // GPC
    w0_of_g = (gidx % GPC) * WPG          # first core-local window of graph

    # per-node: core, local window, slot
    node_rank = np.arange(N) - cum[b]     # position within graph
    w_loc = w0_of_g[b] + node_rank // P   # core-local window 0..WIN-1
    d_loc = node_rank % P
    c_of = core_of_g[b]
    perm = (w_loc // CW) * QR + c_of * (CW * P) + (w_loc % CW) * P + d_loc

    xt = np.zeros((NTOT, x.shape[1]), table_np)
    xt[perm] = x.astype(table_np)

    src = perm[ei[0]]
    dst = perm[ei[1]]
    deg = np.bincount(dst, minlength=NTOT)

    # decompose dst global id -> (core, window, lane)
    def decomp(g):
        q = g // QR
        r = g % QR
        c = r // (CW * P)
        w = q * CW + (r % (CW * P)) // P
        d = r % P
        return c, w, d

    c_d, w_d, d_l = decomp(dst)
    q_s = src // QR
    idx_loc = src % QR

    # recip / mask in (core, lane, window) layout
    recip_pc = np.zeros((NC, P, WIN), np.float32)
    mask_pc = np.zeros((NC, 1, NPC), table_np)
    gl = (np.arange(NTOT) // QR) * 0  # placeholder
    # build from deg via decomposition of all rows
    allg = np.arange(NTOT)
    ac, aw, ad = decomp(allg)
    recip_full = (1.0 / np.maximum(deg, 1)).astype(np.float32)
    mask_full = (deg > 0).astype(np.float32)
    recip_pc[ac, ad, aw] = recip_full
    mask_pc[ac, 0, aw * P + ad] = mask_full.astype(table_np)

    # group-pooled slot layout: per (grp of GRP windows, q) the windows' exact
    # max-over-core counts are packed back to back and only the group total is
    # rounded up to 128. Subtiles may straddle window boundaries; each
    # (subtile, window) segment gets its own dst column / matmul.
    cnt = np.zeros((NC, WIN, NQ), np.int64)
    np.add.at(cnt, (c_d, w_d, q_s), 1)
    M = cnt.max(axis=0)                                  # [WIN, NQ] slots/window
    GRP = 4
    NG = WIN // GRP
    woff = np.zeros((WIN, NQ), np.int64)   # slot offset of w within its (g,q)
    NSUB = np.zeros((NG, NQ), np.int64)    # subtiles per (g,q)
    base = np.zeros((NG, NQ), np.int64)    # first (global) subtile of (g,q)
    segs = {}                              # (g,q) -> [(s_loc, w, col, a, b)]
    run = 0
    nseg = 0
    for q in range(NQ):
        for g in range(NG):
            off = 0
            for w in range(g * GRP, (g + 1) * GRP):
                woff[w, q] = off
                off += int(M[w, q])
            ns = (off + P - 1) // P
            NSUB[g, q] = ns
            base[g, q] = run
            run += ns
            lst = []
            for w in range(g * GRP, (g + 1) * GRP):
                lo = int(woff[w, q])
                hi = lo + int(M[w, q])
                if hi == lo:
                    continue
                for s in range(lo // P, (hi - 1) // P + 1):
                    a = max(lo, s * P)
                    b = min(hi, (s + 1) * P)
                    lst.append((s, w, nseg, a, b))
                    nseg += 1
            segs[(g, q)] = lst
    S_tot = run
    NSEG = nseg

    # sort edges by (core, q_src, window) then assign slots
    key = ((c_d * NQ + q_s) * WIN + w_d)
    order = np.argsort(key, kind="stable")
    s_idx = idx_loc[order]
    s_dl = d_l[order]
    s_key = key[order]
    L = np.bincount(s_key, minlength=NC * NQ * WIN).reshape(NC, NQ, WIN)

    idx_flat = np.zeros((NC, S_tot * P), np.int64)       # pad -> row 0
    dl_flat = np.full((NC, S_tot * P), 255, np.int64)    # pad -> dead dst
    grp_start = np.concatenate([[0], np.cumsum(L.reshape(-1))])
    for c in range(NC):
        for q in range(NQ):
            for w in range(WIN):
                gi = (c * NQ + q) * WIN + w
                n = L[c, q, w]
                if n == 0:
                    continue
                a = grp_start[gi]
                sl0 = int(base[w // GRP, q]) * P + int(woff[w, q])
                idx_flat[c, sl0:sl0 + n] = s_idx[a:a + n]
                dl_flat[c, sl0:sl0 + n] = s_dl[a:a + n]
    assert idx_flat.max() < QR and idx_flat.min() >= 0

    idx16 = np.ascontiguousarray(
        idx_flat.reshape(NC, S_tot * 8, 16).transpose(0, 2, 1)).astype(np.int16)
    idx16 = np.tile(idx16, (1, 8, 1))                    # [NC, 128, S_tot*8]

    # per-segment dst columns [NC, NSEG, P], 255 outside the segment range
    dst_cols = np.full((NC, NSEG, P), 255, np.int64)
    for (g, q), lst in segs.items():
        b0 = int(base[g, q]) * P
        for (s, w, col, a, b) in lst:
            dst_cols[:, col, a - s * P:b - s * P] = dl_flat[:, b0 + a:b0 + b]
    dst_arr = np.ascontiguousarray(
        dst_cols.transpose(0, 2, 1)).astype(ml_dtypes.bfloat16)

    struct = dict(NC=NC, G=G, GPC=GPC, WPG=WPG, WIN=WIN, NPC=NPC, NTOT=NTOT,
                  NQ=NQ, QR=QR, CW=CW, S_tot=S_tot, NSEG=NSEG,
                  NSUB=NSUB, base=base, segs=segs, woff=woff, M=M,
                  GRP=GRP, NG=NG)
    percore = dict(idx16=idx16, dst=dst_arr, recip=recip_pc, mask=mask_pc, xt=xt)
    meta = dict(S_tot=S_tot, WPG=WPG)
    return struct, percore, meta


def build_nc(st, D=128, OUT=2, table_dt=BF16):
    NC, WIN, NPC, NTOT = st["NC"], st["WIN"], st["NPC"], st["NTOT"]
    NQ, QR, CW, GPC, WPG = st["NQ"], st["QR"], st["CW"], st["GPC"], st["WPG"]
    S_tot, NSEG, GRP, NG = st["S_tot"], st["NSEG"], st["GRP"], st["NG"]
    NSUB, base, segs = st["NSUB"], st["base"], st["segs"]
    DT = BF16

    nc = bacc.Bacc("TRN2", target_bir_lowering=False, debug=False,
                   num_devices=NC, num_swdge_queues=4,
                   dynamic_dma_scratch_size=16384)
    xt = nc.dram_tensor("xt", [NTOT, D], table_dt, kind="ExternalInput")
    idx_in = nc.dram_tensor("idx16", [P, S_tot * 8], I16, kind="ExternalInput")
    dst_in = nc.dram_tensor("dstl", [P, NSEG], BF16, kind="ExternalInput")
    recip_in = nc.dram_tensor("recip", [P, WIN], F32, kind="ExternalInput")
    mask_in = nc.dram_tensor("mask", [1, NPC], BF16, kind="ExternalInput")
    w1t_in = nc.dram_tensor("w1t", [D, D], DT, kind="ExternalInput")
    w2t_in = nc.dram_tensor("w2t", [D, D], DT, kind="ExternalInput")
    b1r_in = nc.dram_tensor("b1r", [1, D], DT, kind="ExternalInput")
    b2r_in = nc.dram_tensor("b2r", [1, D], DT, kind="ExternalInput")
    wf1t_in = nc.dram_tensor("wf1t", [D, D], F32, kind="ExternalInput")
    bf1r_in = nc.dram_tensor("bf1r", [1, D], F32, kind="ExternalInput")
    wf2t_in = nc.dram_tensor("wf2t", [D, OUT], F32, kind="ExternalInput")
    bf2r_in = nc.dram_tensor("bf2r", [1, OUT], F32, kind="ExternalInput")
    iota_in = nc.dram_tensor("iota", [P, P], BF16, kind="ExternalInput")
    identb_in = nc.dram_tensor("identb", [P, P], BF16, kind="ExternalInput")
    identf_in = nc.dram_tensor("identf", [P, P], F32, kind="ExternalInput")
    onesg_in = nc.dram_tensor("onesg", [1, NC * GPC], F32, kind="ExternalInput")
    out = nc.dram_tensor("out", [NC * GPC, OUT], F32, kind="ExternalOutput")
    h_loc = nc.dram_tensor("h_loc", [NPC, D], table_dt, kind="Internal")
    htq = [nc.dram_tensor(f"htq{q}", [QR, D], table_dt, kind="Internal",
                          addr_space="Shared") for q in range(NQ)]

    with tile.TileContext(nc) as tc:
        cp = tc.alloc_tile_pool(name="const", bufs=1)
        wp = tc.alloc_tile_pool(name="work", bufs=3)
        mp = tc.alloc_tile_pool(name="msgs", bufs=2)
        ohp = tc.alloc_tile_pool(name="ohp", bufs=2)
        pp_agg = tc.alloc_tile_pool(name="ps_agg", bufs=2, space="PSUM")
        pp_t = tc.alloc_tile_pool(name="ps_t", bufs=2, space="PSUM")
        pp_h = tc.alloc_tile_pool(name="ps_h", bufs=2, space="PSUM")
        pp_p = tc.alloc_tile_pool(name="ps_p", bufs=2, space="PSUM")

        idx_t = cp.tile([P, S_tot * 8], I16)
        nc.sync.dma_start(idx_t[:], idx_in[:])
        dst_t = cp.tile([P, NSEG], BF16)
        nc.sync.dma_start(dst_t[:], dst_in[:])
        recip_t = cp.tile([P, WIN], F32)
        nc.sync.dma_start(recip_t[:], recip_in[:])
        mask_t = cp.tile([1, NPC], BF16)
        nc.sync.dma_start(mask_t[:], mask_in[:])
        w1t_t = cp.tile([D, D], DT)
        nc.sync.dma_start(w1t_t[:], w1t_in[:])
        w2t_t = cp.tile([D, D], DT)
        nc.sync.dma_start(w2t_t[:], w2t_in[:])
        b1r_t = cp.tile([1, D], DT)
        nc.sync.dma_start(b1r_t[:], b1r_in[:])
        b2r_t = cp.tile([1, D], DT)
        nc.sync.dma_start(b2r_t[:], b2r_in[:])
        wf1t_t = cp.tile([D, D], F32)
        nc.sync.dma_start(wf1t_t[:], wf1t_in[:])
        bf1r_t = cp.tile([1, D], F32)
        nc.sync.dma_start(bf1r_t[:], bf1r_in[:])
        wf2t_t = cp.tile([D, OUT], F32)
        nc.sync.dma_start(wf2t_t[:], wf2t_in[:])
        bf2r_t = cp.tile([1, OUT], F32)
        nc.sync.dma_start(bf2r_t[:], bf2r_in[:])
        iota_t = cp.tile([P, P], BF16)
        nc.sync.dma_start(iota_t[:], iota_in[:])
        identb_t = cp.tile([P, P], BF16)
        nc.sync.dma_start(identb_t[:], identb_in[:])
        identf_t = cp.tile([P, P], F32)
        nc.sync.dma_start(identf_t[:], identf_in[:])
        onesg_t = cp.tile([1, NC * GPC], F32)
        nc.sync.dma_start(onesg_t[:], onesg_in[:])
        poolT = cp.tile([P, GPC], F32)
        nc.vector.memset(poolT[:], 0)
        agg_sb = cp.tile([P, WIN * D], F32)  # f32 accumulator, all windows

        Sg_max = int(NSUB.max())
        MAXSUB = 8  # <=1024 idx per call (swdge scratch limit)
        OHMAX = max(len(lst) for lst in segs.values() if lst)
        slabno = 0

        callno = 0
        for layer in range(2):
            wt = w1t_t if layer == 0 else w2t_t
            br = b1r_t if layer == 0 else b2r_t
            nc.vector.memset(agg_sb[:], 0)
            next_ag = 0
            for g in range(NG):
                ws = list(range(g * GRP, (g + 1) * GRP))
                for q in range(NQ):
                    table = xt[q * QR:(q + 1) * QR, :] if layer == 0 else htq[q][:, :]
                    Sg = int(NSUB[g, q])
                    if Sg == 0:
                        continue
                    base_sub = int(base[g, q])
                    seglist = segs[(g, q)]
                    col0 = seglist[0][2]
                    nsg = len(seglist)
                    ncall = (Sg + MAXSUB - 1) // MAXSUB
                    lo_n = Sg // ncall
                    sizes = [lo_n + (1 if j < Sg % ncall else 0)
                             for j in range(ncall)]
                    tiles = []
                    off = 0
                    for j, n in enumerate(sizes):
                        mt = mp.tile([P, n * D], table_dt,
                                     tag=f"msgs{callno % 8}",
                                     name=f"m{layer}_{q}_{g}_{j}",
                                     padded_shape=[P, MAXSUB * D])
                        tiles.append((mt, off, n))
                        c0 = (base_sub + off) * 8
                        nc.gpsimd.dma_gather(
                            out_ap=mt[:].rearrange("p (s d) -> p s d", d=D),
                            in_ap=table,
                            idxs_ap=idx_t[:, c0: c0 + n * 8],
                            num_idxs=n * P,
                            num_idxs_reg=n * P,
                            elem_size=D,
                            queue_num=callno % 4,
                        )
                        callno += 1
                        off += n

                    def m3_of(s):
                        for mt, o, n in tiles:
                            if o <= s < o + n:
                                return mt[:].rearrange(
                                    "p (s d) -> p s d", d=D)[:, s - o, :]
                        raise AssertionError(s)

                    # one wide is_equal builds all of this (g,q)'s one-hot
                    # segment matrices at once
                    ohslab = ohp.tile([P, nsg * P], BF16,
                                      tag=f"ohs{slabno % 2}",
                                      name=f"oh{layer}_{q}_{g}",
                                      padded_shape=[P, OHMAX * P])
                    slabno += 1
                    nc.vector.tensor_tensor(
                        out=ohslab[:].rearrange("p (n d) -> p n d", d=P),
                        in0=dst_t[:, col0:col0 + nsg].rearrange(
                            "p (n o) -> p n o", o=1).to_broadcast([P, nsg, P]),
                        in1=iota_t[:].rearrange(
                            "p (o d) -> p o d", o=1).to_broadcast([P, nsg, P]),
                        op=mybir.AluOpType.is_equal,
                    )

                    for w in ws:
                        segl = [t for t in seglist if t[1] == w]
                        if not segl:
                            continue
                        ps = pp_agg.tile([P, D], F32, tag="agg")
                        for k, (s, _w, col, a, b) in enumerate(segl):
                            kc = col - col0
                            nc.tensor.matmul(ps[:],
                                             lhsT=ohslab[:, kc * P:(kc + 1) * P],
                                             rhs=m3_of(s),
                                             start=(k == 0),
                                             stop=(k == len(segl) - 1))
                        aslice = agg_sb[:, w * D:(w + 1) * D]
                        nc.vector.tensor_tensor(out=aslice, in0=ps[:], in1=aslice,
                                                op=mybir.AluOpType.add)

                # transforms for this group's windows (all quads aggregated);
                # layer0 additionally fires AllGather chunks as soon as all
                # windows of a chunk have been written to h_loc.
                for w in ws:
                    agg_s = wp.tile([P, D], BF16, tag="aggs")
                    nc.vector.tensor_scalar(out=agg_s[:],
                                            in0=agg_sb[:, w * D:(w + 1) * D],
                                            scalar1=recip_t[:, w:w + 1],
                                            scalar2=None,
                                            op0=mybir.AluOpType.mult)
                    aggT_p = pp_t.tile([P, D], BF16, tag="aggT")
                    nc.tensor.transpose(aggT_p[:], agg_s[:], identb_t[:])
                    aggT_s = wp.tile([P, D], BF16, tag="aggTs")
                    nc.scalar.activation(aggT_s[:], aggT_p[:],
                                         mybir.ActivationFunctionType.Copy)
                    h_p = pp_h.tile([P, D], F32, tag="h")
                    nc.tensor.matmul(h_p[:], lhsT=aggT_s[:], rhs=wt[:],
                                     start=True, stop=False)
                    nc.tensor.matmul(h_p[:], lhsT=mask_t[:1, w * P:(w + 1) * P],
                                     rhs=br[:], start=False, stop=True)
                    h_s = wp.tile([P, D], table_dt, tag="hs")
                    nc.scalar.activation(h_s[:], h_p[:],
                                         mybir.ActivationFunctionType.Relu)
                    if layer == 0:
                        nc.sync.dma_start(h_loc[w * P:(w + 1) * P, :], h_s[:])
                    else:
                        lg = w // WPG
                        hT_p = pp_p.tile([P, P], BF16, tag="hT")
                        nc.tensor.transpose(hT_p[:], h_s[:], identb_t[:])
                        wmax = wp.tile([P, 1], F32, tag="wmax")
                        nc.vector.reduce_max(wmax[:], hT_p[:],
                                             axis=mybir.AxisListType.X)
                        nc.vector.tensor_tensor(out=poolT[:, lg:lg + 1],
                                                in0=wmax[:],
                                                in1=poolT[:, lg:lg + 1],
                                                op=mybir.AluOpType.max)
                if layer == 0:
                    while (next_ag < NCHUNK
                           and ws[-1] >= (next_ag + 1) * CW - 1):
                        nc.gpsimd.collective_compute(
                            "AllGather", mybir.AluOpType.bypass,
                            replica_groups=[list(range(NC))],
                            ins=[h_loc[next_ag * CW * P:(next_ag + 1) * CW * P, :]],
                            outs=[htq[next_ag][:]],
                        )
                        next_ag += 1

        # ---- head ----
        NGr = NC * GPC
        dp = tc.alloc_tile_pool(name="dram", bufs=1, space="DRAM")
        pag_in = dp.tile([P, GPC], F32)
        pag_out = dp.tile([NC * P, GPC], F32, addr_space="Shared")
        nc.sync.dma_start(pag_in[:], poolT[:])
        nc.gpsimd.collective_compute(
            "AllGather", mybir.AluOpType.bypass,
            replica_groups=[list(range(NC))],
            ins=[pag_in[:]], outs=[pag_out[:]],
        )
        pall = cp.tile([P, NGr], F32)
        pr = pag_out[:].rearrange("(c p) g -> p c g", c=NC)
        for c in range(NC):
            nc.sync.dma_start(pall[:, c * GPC:(c + 1) * GPC], pr[:, c, :])
        z_p = pp_agg.tile([P, NGr], F32, tag="agg")
        nc.tensor.matmul(z_p[:], lhsT=wf1t_t[:], rhs=pall[:], start=True, stop=False)
        nc.tensor.matmul(z_p[:], lhsT=bf1r_t[:1, :], rhs=onesg_t[:1, :],
                         start=False, stop=True)
        zr = wp.tile([P, NGr], F32, tag="zr")
        nc.scalar.activation(zr[:], z_p[:], mybir.ActivationFunctionType.Relu)
        z2_p = pp_h.tile([OUT, NGr], F32, tag="h")
        nc.tensor.matmul(z2_p[:], lhsT=wf2t_t[:], rhs=zr[:], start=True, stop=False)
        nc.tensor.matmul(z2_p[:], lhsT=bf2r_t[:1, :], rhs=onesg_t[:1, :],
                         start=False, stop=True)
        z2 = wp.tile([OUT, NGr], F32, tag="z2")
        nc.vector.tensor_copy(z2[:], z2_p[:])
        zt_p = pp_t.tile([NGr, OUT], F32, tag="aggT")
        nc.tensor.transpose(zt_p[:], z2[:], identf_t[:OUT, :OUT])
        zt = wp.tile([NGr, OUT], F32, tag="zt")
        nc.vector.tensor_copy(zt[:], zt_p[:])
        mx = wp.tile([NGr, 1], F32, tag="mx")
        nc.vector.reduce_max(mx[:], zt[:], axis=mybir.AxisListType.X)
        zs = wp.tile([NGr, OUT], F32, tag="zs")
        nc.vector.tensor_scalar(out=zs[:], in0=zt[:], scalar1=mx[:], scalar2=None,
                                op0=mybir.AluOpType.subtract)
        ex = wp.tile([NGr, OUT], F32, tag="ex")
        nc.scalar.activation(ex[:], zs[:], mybir.ActivationFunctionType.Exp)
        sm = wp.tile([NGr, 1], F32, tag="sm")
        nc.vector.reduce_sum(sm[:], ex[:], axis=mybir.AxisListType.X)
        lg_ = wp.tile([NGr, 1], F32, tag="lg")
        nc.scalar.activation(lg_[:], sm[:], mybir.ActivationFunctionType.Ln)
        logz = wp.tile([NGr, 1], F32, tag="logz")
        nc.vector.tensor_tensor(out=logz[:], in0=mx[:], in1=lg_[:],
                                op=mybir.AluOpType.add)
        res = wp.tile([NGr, OUT], F32, tag="res")
        nc.vector.tensor_scalar(out=res[:], in0=zt[:], scalar1=logz[:], scalar2=None,
                                op0=mybir.AluOpType.subtract)
        nc.sync.dma_start(out[:], res[:])

        for p_ in (dp, pp_p, pp_h, pp_t, pp_agg, ohp, mp, wp, cp):
            p_.release()
    nc.compile()
    return nc


def make_inputs(st, percore, W1, b1, W2, b2, Wf1, bf1, Wf2, bf2):
    NC, GPC = st["NC"], st["GPC"]
    bf = ml_dtypes.bfloat16
    iota = np.broadcast_to(np.arange(P, dtype=np.float32), (P, P)).astype(bf)
    ident = np.eye(P, dtype=np.float32)
    common = dict(
        xt=percore["xt"],
        w1t=np.ascontiguousarray(np.asarray(W1, np.float32).T).astype(bf),
        w2t=np.ascontiguousarray(np.asarray(W2, np.float32).T).astype(bf),
        b1r=np.asarray(b1, np.float32)[None, :].astype(bf),
        b2r=np.asarray(b2, np.float32)[None, :].astype(bf),
        wf1t=np.ascontiguousarray(np.asarray(Wf1, np.float32).T),
        bf1r=np.asarray(bf1, np.float32)[None, :],
        wf2t=np.ascontiguousarray(np.asarray(Wf2, np.float32).T),
        bf2r=np.asarray(bf2, np.float32)[None, :],
        iota=np.ascontiguousarray(iota),
        identb=ident.astype(bf),
        identf=ident,
        onesg=np.ones((1, NC * GPC), np.float32),
    )
    in_maps = []
    for c in range(NC):
        m = dict(common)
        m["idx16"] = np.ascontiguousarray(percore["idx16"][c])
        m["dstl"] = np.ascontiguousarray(percore["dst"][c])
        m["recip"] = np.ascontiguousarray(percore["recip"][c])
        m["mask"] = np.ascontiguousarray(percore["mask"][c])
        in_maps.append(m)
    return in_maps


_CACHE = {}


def kernel(**inputs):
    """Full-input GNN kernel: shards across 8 NeuronCores internally."""
    import os
    x = np.asarray(inputs["x"], np.float32)
    ei = np.asarray(inputs["edge_index"])
    batch = np.asarray(inputs["batch"])
    st, percore, _meta = preprocess(x, ei, batch)
    key = (st["WIN"], st["NPC"], st["S_tot"], st["NQ"])
    if key not in _CACHE:
        _CACHE[key] = build_nc(st)
    nc = _CACHE[key]
    in_maps = make_inputs(st, percore,
                          inputs["W1"], inputs["b1"], inputs["W2"], inputs["b2"],
                          inputs["Wf1"], inputs["bf1"], inputs["Wf2"], inputs["bf2"])
    trace = os.environ.get("GNN_TRACE", "0") == "1"
    res = run_bass_kernel_spmd(nc, in_maps, core_ids=list(range(st["NC"])), trace=trace)
    global LAST_EXEC_NS, LAST_TRACE
    LAST_EXEC_NS = res.exec_time_ns
    LAST_TRACE = res.instructions_and_trace[1] if res.instructions_and_trace else None
    return np.asarray(res.results[0]["out"], np.float32)


LAST_EXEC_NS = None
LAST_TRACE = None
